# revision 23
# baseline (speedup 1.0000x reference)
"""Bass/Tile TRN2 kernel for the sparse-attention (pointer-generator style)
attention module.

Reference computation (B=32, L=2048, N=1024):
    s         = s_t_hat @ W_dec.T + b_dec                     [B, N]
    attn_feat = tanh(enc_feat + s[:, None, :] + cov[..., None] * W_c)
    e         = einsum('bln,n->bl', attn_feat, W_v)           [B, L]
    attn      = softmax(e, axis=1) * mask; attn /= attn.sum(1) + 1e-12
    context   = einsum('bl,bln->bn', attn, h)                 [B, N]
    cov_new   = cov + attn

Strategy: pure data-parallel over batch (4 batches per core, 8 cores), no
collectives.  Memory-bound: each core streams its enc_feat (fp16) and h
(two bf16 streams: h_hi + h_lo residual) through SBUF in 1 MiB DMAs, with
pass C (context) interleaved per batch behind pass B (scores).

Numerics: PE fp32 matmuls run ~8x slower per column than bf16/fp16, so all
matmuls are 16-bit with error control:
  - terms matmul (1*s + cov*W_c) in bf16: perturbs the tanh argument by
    ~1e-5 absolute; softmax only sees absolute-e error ~1e-8.
  - enc in fp16: absolute-e error ~1e-6.
  - the e-dot runs on DVE in fp16 with W_v pre-scaled by 2^10 (keeps it
    in fp16 normal range); the 2^-10 is folded into the Exp scale.
  - context = a_hi@h_hi + a_lo@h_hi + a_hi@h_lo with bf16 hi/lo splits of
    both attn and h: exact to ~2^-17 per factor, fp32 PSUM accumulation.
"""

import ml_dtypes
import numpy as np

import concourse.bacc as bacc
import concourse.bass as bass
import concourse.bass_isa as bass_isa
import concourse.mybir as mybir
import concourse.tile as tile
from concourse.bass_utils import run_bass_kernel_spmd

B, L, N = 32, 2048, 1024
M = 8            # cores
BL = B // M      # local batches per core (4)
P = 128          # SBUF partitions
NCH = L // P     # L-chunks per batch (16)
Q = 4            # L-chunks per enc streaming DMA (1 MiB fp16)
NT = NCH // Q    # enc streaming DMAs per batch (4)
QH = 4           # L-chunks per h streaming DMA (1 MiB bf16)
NTH = NCH // QH  # h streaming DMAs per batch per half (4)
WV_SCALE = 1024.0
F32 = mybir.dt.float32
F16 = mybir.dt.float16
BF16 = mybir.dt.bfloat16
AF = mybir.ActivationFunctionType
ALU = mybir.AluOpType
NPBF16 = ml_dtypes.bfloat16

_CACHED_NC = None


def _build_nc():
    nc = bacc.Bacc()

    hh_d = nc.declare_dram_parameter("h_hi", [BL, L, N], BF16, isOutput=False)
    hl_d = nc.declare_dram_parameter("h_lo", [BL, L, N], BF16, isOutput=False)
    enc_d = nc.declare_dram_parameter("enc", [BL, L, N], F16, isOutput=False)
    lhst_d = nc.declare_dram_parameter("lhst", [2, BL * L], BF16, isOutput=False)
    covs_d = nc.declare_dram_parameter("cov_swz", [P, BL * NCH], F32, isOutput=False)
    mask_d = nc.declare_dram_parameter("mask_swz", [P, BL * NCH], F32, isOutput=False)
    stT_d = nc.declare_dram_parameter("stT", [N, BL], BF16, isOutput=False)
    wdecT_d = nc.declare_dram_parameter("wdecT", [N, N], BF16, isOutput=False)
    bdec_d = nc.declare_dram_parameter("b_dec", [1, N], F32, isOutput=False)
    wc_d = nc.declare_dram_parameter("w_c", [1, BL * N], BF16, isOutput=False)
    wv_d = nc.declare_dram_parameter("w_v", [1, N], F16, isOutput=False)

    attn_o = nc.declare_dram_parameter("attn_swz", [P, BL * NCH], F32, isOutput=True)
    ctx_o = nc.declare_dram_parameter("ctx", [BL, N], F32, isOutput=True)
    covn_o = nc.declare_dram_parameter("covnew_swz", [P, BL * NCH], F32, isOutput=True)

    # Stream views: chunk t covers L rows [t*Q*P, (t+1)*Q*P); partition p of
    # the tile holds rows t*Q*P + q*P + p for q in range(Q).
    enc_r = enc_d[:].rearrange("b (t q p) n -> b t p q n", q=Q, p=P)
    hh_r = hh_d[:].rearrange("b (t q p) n -> b t p q n", q=QH, p=P)
    hl_r = hl_d[:].rearrange("b (t q p) n -> b t p q n", q=QH, p=P)

    with tile.TileContext(nc) as tc:
        with (
            tc.tile_pool(name="singles", bufs=1) as singles,
            tc.tile_pool(name="wdec_pool", bufs=4) as wdec_pool,
            tc.tile_pool(name="stream", bufs=4) as stream,
            tc.tile_pool(name="hstream", bufs=4) as hstream,
            tc.tile_pool(name="sums", bufs=3) as sums,
            tc.tile_pool(name="tanhs", bufs=3) as tanhs,
            tc.tile_pool(name="smalls", bufs=8) as smalls,
            tc.tile_pool(name="ctxs", bufs=1) as ctxs,
            tc.tile_pool(name="scrp", bufs=3) as scrp,
            tc.tile_pool(name="ps_big", bufs=1, space="PSUM") as ps_big,
            tc.tile_pool(name="ps_terms", bufs=2, space="PSUM") as ps_terms,
            tc.tile_pool(name="ps_tot", bufs=2, space="PSUM") as ps_tot,
        ):
            # ---------------- setup ----------------
            # NOTE: all DMAs go through HWDGE (nc.sync / nc.scalar) — SWDGE
            # (gpsimd) descriptor generation deadlocks against concurrent DVE
            # activity on this part (hardware port-sharing hazard).
            wv_b = singles.tile([P, N], F16)
            nc.sync.dma_start(
                out=wv_b[:],
                in_=bass.AP(tensor=wv_d, offset=0, ap=[[0, P], [1, N]]),
            )
            bdec_b = singles.tile([BL, N], F32)
            nc.sync.dma_start(
                out=bdec_b[:],
                in_=bass.AP(tensor=bdec_d, offset=0, ap=[[0, BL], [1, N]]),
            )

            # lhsT for the rank-2 "terms" matmul: partition 0 = ones,
            # partition 1 = coverage rows (original L order); host-prepared.
            lhsT_cov = singles.tile([2, BL, L], BF16)
            nc.sync.dma_start(
                out=lhsT_cov[:],
                in_=lhst_d[:].rearrange("two (b l) -> two b l", b=BL),
            )

            # rhs for the terms matmul: partition 0 = s (per batch),
            # partition 1 = W_c (host-replicated per batch).  bf16.
            rhs_sw = singles.tile([2, BL, N], BF16)
            nc.sync.dma_start(
                out=rhs_sw[1:2, :, :],
                in_=wc_d[:].rearrange("one (b n) -> one b n", b=BL),
            )

            mask_t = singles.tile([P, BL * NCH], F32)
            nc.sync.dma_start(out=mask_t[:], in_=mask_d[:])
            covs_t = singles.tile([P, BL * NCH], F32)
            nc.sync.dma_start(out=covs_t[:], in_=covs_d[:])

            e_t = singles.tile([P, BL * NCH], F32)
            attn_t = singles.tile([P, BL * NCH], F32)
            # a2[:, col, 0] = bf16(attn), a2[:, col, 1] = bf16 residual —
            # an [128, 2] slice is the lhsT of the merged context matmul
            a2 = singles.tile([P, BL * NCH, 2], BF16)
            attn_lof = singles.tile([P, BL * NCH], F32)
            covn_t = singles.tile([P, BL * NCH], F32)

            e_scr = singles.tile([P, N], F16)  # dead output of the dot-accum
            ones_col = singles.tile([P, 1], F32)
            nc.vector.memset(ones_col[:], 1.0)
            ones_row = singles.tile([1, P], F32)
            nc.vector.memset(ones_row[:], 1.0)

            # ---------------- dec_proj: s = s_t_hat @ W_dec.T + b_dec -------
            s_ps = ps_big.tile([BL, N], F32, tag="big")
            for kb in range(N // P):
                stT_t = smalls.tile([P, BL], BF16, tag="stT")
                nc.sync.dma_start(out=stT_t[:], in_=stT_d[kb * P : (kb + 1) * P, :])
                wdecT_t = wdec_pool.tile([P, N], BF16)
                nc.sync.dma_start(
                    out=wdecT_t[:], in_=wdecT_d[kb * P : (kb + 1) * P, :]
                )
                for nh in range(2):
                    nc.tensor.matmul(
                        s_ps[:, nh * 512 : (nh + 1) * 512],
                        stT_t[:],
                        wdecT_t[:, nh * 512 : (nh + 1) * 512],
                        start=(kb == 0),
                        stop=(kb == N // P - 1),
                    )
            s_sb = singles.tile([BL, N], F32)
            nc.vector.tensor_add(s_sb[:], s_ps[:], bdec_b[:])
            s_bf = singles.tile([BL, N], BF16)
            nc.vector.tensor_copy(s_bf[:], s_sb[:])
            for b in range(BL):
                # cross-partition move b -> 0 into the rhs tile
                nc.sync.dma_start(out=rhs_sw[0:1, b, :], in_=s_bf[b : b + 1, :])

            # ------------- staggered pipeline over local batches -------------
            # Stage s runs pass B of batch s interleaved (at the t-step
            # level) with pass C of batch s-1, so the DVE-bound score pass
            # and the PE-bound context pass overlap and the two DMA streams
            # advance together.
            ctx_ps = None
            for s in range(BL + 1):
                bB = s            # batch for pass B this stage
                bC = s - 1        # batch for pass C this stage
                if bC >= 0:
                    ctx_ps = ps_big.tile([2, N], F32, tag="big")
                pend_th = None
                for t in range(NT):
                    if bB < BL:
                        enc_t = stream.tile([P, Q, N], F16, tag="stream")
                        dme = nc.sync if t % 2 == 0 else nc.scalar
                        dme.dma_start(out=enc_t[:], in_=enc_r[bB, t])
                        su = sums.tile([P, Q, N], F16, tag="su")
                        for q in range(Q):
                            c = t * Q + q
                            terms = ps_terms.tile([P, N], F32, tag="terms")
                            for nh in range(2):
                                nc.tensor.matmul(
                                    terms[:, nh * 512 : (nh + 1) * 512],
                                    lhsT_cov[:, bB, c * P : (c + 1) * P],
                                    rhs_sw[:, bB, nh * 512 : (nh + 1) * 512],
                                    start=True,
                                    stop=True,
                                )
                            nc.vector.tensor_add(
                                su[:, q, :], enc_t[:, q, :], terms[:]
                            )
                        th = tanhs.tile([P, Q, N], F16, tag="th")
                        nc.scalar.activation(th[:], su[:], AF.Tanh)
                        # dots for the PREVIOUS tile: keeps DVE busy during
                        # this tile's tanh latency (engine order is fixed at
                        # schedule time, so emit adds(t) before dots(t-1))
                        if pend_th is not None:
                            pth, pt = pend_th
                            for q in range(Q):
                                c = pt * Q + q
                                scr = scrp.tile([P, N], F16, tag="scr")
                                nc.gpsimd.tensor_mul(scr[:], pth[:, q, :], wv_b[:])
                                nc.vector.tensor_scalar(
                                    out=e_scr[:],
                                    in0=scr[:],
                                    scalar1=1.0,
                                    scalar2=0.0,
                                    op0=ALU.mult,
                                    op1=ALU.add,
                                    accum_out=e_t[:, bB * NCH + c : bB * NCH + c + 1],
                                )
                        pend_th = (th, t)

                    if bC >= 0:
                        # pass C t-step for batch bC (QH == Q so t aligns)
                        hh_t = hstream.tile([P, QH, N], BF16, tag="hstream")
                        nc.sync.dma_start(out=hh_t[:], in_=hh_r[bC, t])
                        hl_t = hstream.tile([P, QH, N], BF16, tag="hstream")
                        nc.sync.dma_start(out=hl_t[:], in_=hl_r[bC, t])
                        for q in range(QH):
                            c = t * QH + q
                            col = bC * NCH + c
                            for hi, h_til in enumerate((hh_t, hl_t)):
                                for nh in range(2):
                                    nsl = slice(nh * 512, (nh + 1) * 512)
                                    nc.tensor.matmul(
                                        ctx_ps[:, nsl],
                                        a2[:, col, :],
                                        h_til[:, q, nsl],
                                        start=(c == 0 and hi == 0),
                                        stop=(c == NCH - 1 and hi == 1),
                                        skip_group_check=True,
                                    )

                if bC >= 0:
                    # finalize context for bC: rows are hi/lo partial sums
                    ctx_sb = ctxs.tile([2, N], F32, tag="ctx")
                    nc.vector.tensor_copy(ctx_sb[:], ctx_ps[:])
                    ctx_red = ctxs.tile([2, N], F32, tag="ctxred")
                    nc.gpsimd.partition_all_reduce(
                        ctx_red[:],
                        ctx_sb[:],
                        channels=2,
                        reduce_op=bass_isa.ReduceOp.add,
                    )
                    nc.sync.dma_start(out=ctx_o[bC : bC + 1, :], in_=ctx_red[0:1, :])

                if bB < BL:
                    # flush the last tile's pending dots
                    if pend_th is not None:
                        pth, pt = pend_th
                        for q in range(Q):
                            c = pt * Q + q
                            scr = scrp.tile([P, N], F16, tag="scr")
                            nc.gpsimd.tensor_mul(scr[:], pth[:, q, :], wv_b[:])
                            nc.vector.tensor_scalar(
                                out=e_scr[:],
                                in0=scr[:],
                                scalar1=1.0,
                                scalar2=0.0,
                                op0=ALU.mult,
                                op1=ALU.add,
                                accum_out=e_t[:, bB * NCH + c : bB * NCH + c + 1],
                            )
                        pend_th = None
                    # ---- masked softmax + renorm + coverage update for bB ----
                    sl = slice(bB * NCH, (bB + 1) * NCH)
                    pexp = smalls.tile([P, NCH], F32, tag="pexp")
                    # e was accumulated against W_v * 2^10; undo via Exp scale
                    nc.scalar.activation(
                        pexp[:], e_t[:, sl], AF.Exp, scale=1.0 / WV_SCALE
                    )
                    pm = smalls.tile([P, NCH], F32, tag="pm")
                    partial = smalls.tile([P, 1], F32, tag="partial")
                    nc.vector.scalar_tensor_tensor(
                        out=pm[:],
                        in0=pexp[:],
                        scalar=1.0,
                        in1=mask_t[:, sl],
                        op0=ALU.mult,
                        op1=ALU.mult,
                        accum_out=partial[:],
                    )
                    # partition-sum of `partial` + broadcast via two tiny
                    # fp32 matmuls on a dedicated PSUM bank
                    tot1 = ps_tot.tile([1, 1], F32, tag="tot")
                    nc.tensor.matmul(
                        tot1[:], partial[:], ones_col[:], start=True, stop=True
                    )
                    tot1_sb = smalls.tile([1, 1], F32, tag="tot1")
                    nc.vector.tensor_copy(tot1_sb[:], tot1[:])
                    tot2 = ps_tot.tile([P, 1], F32, tag="tot")
                    nc.tensor.matmul(
                        tot2[:], ones_row[:], tot1_sb[:], start=True, stop=True
                    )
                    rtot = smalls.tile([P, 1], F32, tag="rtot")
                    nc.vector.reciprocal(rtot[:], tot2[:])
                    nc.vector.tensor_scalar_mul(attn_t[:, sl], pm[:], rtot[:])
                    nc.vector.tensor_add(
                        covn_t[:, sl], covs_t[:, sl], attn_t[:, sl]
                    )
                    # attn hi/lo bf16 split for the context matmul
                    nc.vector.tensor_copy(a2[:, sl, 0], attn_t[:, sl])
                    nc.vector.tensor_sub(
                        attn_lof[:, sl], attn_t[:, sl], a2[:, sl, 0]
                    )
                    nc.vector.tensor_copy(a2[:, sl, 1], attn_lof[:, sl])
                    # stream per-batch outputs out as soon as they're ready
                    nc.sync.dma_start(out=attn_o[:, sl], in_=attn_t[:, sl])
                    nc.sync.dma_start(out=covn_o[:, sl], in_=covn_t[:, sl])

    nc.finalize()
    return nc


def _swz(x):
    """[BL, L] -> [P, BL*NCH]: column b*NCH+c, partition p <- x[b, c*P+p]."""
    return np.ascontiguousarray(
        x.reshape(BL, NCH, P).transpose(2, 0, 1).reshape(P, BL * NCH)
    )


def _unswz(y):
    """inverse of _swz"""
    return np.ascontiguousarray(
        y.reshape(P, BL, NCH).transpose(1, 2, 0).reshape(BL, L)
    )


def build_in_maps(inputs):
    return _build_in_maps(**inputs)


def _build_in_maps(h, enc_feat, attn_mask, s_t_hat, coverage, W_dec, b_dec, W_c, W_v):
    h = np.asarray(h, np.float32)
    enc_feat = np.asarray(enc_feat, np.float32)
    attn_mask = np.asarray(attn_mask, np.float32)
    s_t_hat = np.asarray(s_t_hat, np.float32)
    coverage = np.asarray(coverage, np.float32)
    h_hi = h.astype(NPBF16)
    h_lo = (h - h_hi.astype(np.float32)).astype(NPBF16)
    wdecT = np.ascontiguousarray(np.asarray(W_dec, np.float32).T).astype(NPBF16)
    bdec = np.ascontiguousarray(np.asarray(b_dec, np.float32).reshape(1, N))
    wc = np.tile(
        np.asarray(W_c, np.float32).reshape(1, N), (1, BL)
    ).reshape(1, BL * N).astype(NPBF16)
    wv = (np.asarray(W_v, np.float32).reshape(1, N) * WV_SCALE).astype(np.float16)

    in_maps = []
    for core in range(M):
        sl = slice(core * BL, (core + 1) * BL)
        cov = coverage[sl]
        lhst = np.concatenate(
            [np.ones((1, BL * L), np.float32), cov.reshape(1, BL * L)]
        ).astype(NPBF16)
        in_maps.append(
            {
                "h_hi": np.ascontiguousarray(h_hi[sl]),
                "h_lo": np.ascontiguousarray(h_lo[sl]),
                "enc": np.ascontiguousarray(enc_feat[sl].astype(np.float16)),
                "lhst": lhst,
                "cov_swz": _swz(cov),
                "mask_swz": _swz(attn_mask[sl]),
                "stT": np.ascontiguousarray(s_t_hat[sl].T.astype(NPBF16)),
                "wdecT": wdecT,
                "b_dec": bdec,
                "w_c": wc,
                "w_v": wv,
            }
        )
    return in_maps


def kernel(**inputs):
    global _CACHED_NC
    in_maps = build_in_maps(inputs)
    if _CACHED_NC is None:
        _CACHED_NC = _build_nc()
    res = run_bass_kernel_spmd(_CACHED_NC, in_maps, list(range(M)))

    attn = np.empty((B, L), np.float32)
    ctx = np.empty((B, N), np.float32)
    covn = np.empty((B, L), np.float32)
    for core in range(M):
        r = res.results[core]
        sl = slice(core * BL, (core + 1) * BL)
        attn[sl] = _unswz(r["attn_swz"])
        ctx[sl] = r["ctx"]
        covn[sl] = _unswz(r["covnew_swz"])
    return attn, ctx, covn


# revision 24
# speedup vs baseline: 1.1442x; 1.1442x over previous
"""Bass/Tile TRN2 kernel for the sparse-attention (pointer-generator style)
attention module.

Reference computation (B=32, L=2048, N=1024):
    s         = s_t_hat @ W_dec.T + b_dec                     [B, N]
    attn_feat = tanh(enc_feat + s[:, None, :] + cov[..., None] * W_c)
    e         = einsum('bln,n->bl', attn_feat, W_v)           [B, L]
    attn      = softmax(e, axis=1) * mask; attn /= attn.sum(1) + 1e-12
    context   = einsum('bl,bln->bn', attn, h)                 [B, N]
    cov_new   = cov + attn

Strategy: pure data-parallel over batch (4 batches per core, 8 cores), no
collectives.  Memory-bound: each core streams its enc_feat (fp16) and h
(two bf16 streams: h_hi + h_lo residual) through SBUF in 1 MiB DMAs, with
pass C (context) interleaved per batch behind pass B (scores).

Numerics: PE fp32 matmuls run ~8x slower per column than bf16/fp16, so all
matmuls are 16-bit with error control:
  - terms matmul (1*s + cov*W_c) in bf16: perturbs the tanh argument by
    ~1e-5 absolute; softmax only sees absolute-e error ~1e-8.
  - enc in fp16: absolute-e error ~1e-6.
  - the e-dot runs on DVE in fp16 with W_v pre-scaled by 2^10 (keeps it
    in fp16 normal range); the 2^-10 is folded into the Exp scale.
  - context = a_hi@h_hi + a_lo@h_hi + a_hi@h_lo with bf16 hi/lo splits of
    both attn and h: exact to ~2^-17 per factor, fp32 PSUM accumulation.
"""

import ml_dtypes
import numpy as np

import concourse.bacc as bacc
import concourse.bass as bass
import concourse.bass_isa as bass_isa
import concourse.mybir as mybir
import concourse.tile as tile
from concourse.bass_utils import run_bass_kernel_spmd

B, L, N = 32, 2048, 1024
M = 8            # cores
BL = B // M      # local batches per core (4)
P = 128          # SBUF partitions
NCH = L // P     # L-chunks per batch (16)
Q = 4            # L-chunks per enc streaming DMA (1 MiB fp16)
NT = NCH // Q    # enc streaming DMAs per batch (4)
QH = 4           # L-chunks per h streaming DMA (1 MiB bf16)
NTH = NCH // QH  # h streaming DMAs per batch per half (4)
WV_SCALE = 1024.0
F32 = mybir.dt.float32
F16 = mybir.dt.float16
BF16 = mybir.dt.bfloat16
AF = mybir.ActivationFunctionType
ALU = mybir.AluOpType
NPBF16 = ml_dtypes.bfloat16

_CACHED_NC = None


def _build_nc():
    nc = bacc.Bacc()

    hh_d = nc.declare_dram_parameter("h_hi", [BL, L, N], BF16, isOutput=False)
    hl_d = nc.declare_dram_parameter("h_lo", [BL, L, N], BF16, isOutput=False)
    enc_d = nc.declare_dram_parameter("enc", [BL, L, N], F16, isOutput=False)
    lhst_d = nc.declare_dram_parameter("lhst", [2, BL * L], BF16, isOutput=False)
    covs_d = nc.declare_dram_parameter("cov_swz", [P, BL * NCH], F32, isOutput=False)
    mask_d = nc.declare_dram_parameter("mask_swz", [P, BL * NCH], F32, isOutput=False)
    stT_d = nc.declare_dram_parameter("stT", [N, BL], BF16, isOutput=False)
    wdecT_d = nc.declare_dram_parameter("wdecT", [N, N], BF16, isOutput=False)
    bdec_d = nc.declare_dram_parameter("b_dec", [1, N], F32, isOutput=False)
    wc_d = nc.declare_dram_parameter("w_c", [1, BL * N], BF16, isOutput=False)
    wv_d = nc.declare_dram_parameter("w_v", [1, N], F16, isOutput=False)

    attn_o = nc.declare_dram_parameter("attn_swz", [P, BL * NCH], F32, isOutput=True)
    ctx_o = nc.declare_dram_parameter("ctx", [BL, N], F32, isOutput=True)
    covn_o = nc.declare_dram_parameter("covnew_swz", [P, BL * NCH], F32, isOutput=True)

    # Stream views: chunk t covers L rows [t*Q*P, (t+1)*Q*P); partition p of
    # the tile holds rows t*Q*P + q*P + p for q in range(Q).
    enc_r = enc_d[:].rearrange("b (t q p) n -> b t p q n", q=Q, p=P)
    hh_r = hh_d[:].rearrange("b (t q p) n -> b t p q n", q=QH, p=P)
    hl_r = hl_d[:].rearrange("b (t q p) n -> b t p q n", q=QH, p=P)

    with tile.TileContext(nc) as tc:
        with (
            tc.tile_pool(name="singles", bufs=1) as singles,
            tc.tile_pool(name="wdec_pool", bufs=4) as wdec_pool,
            tc.tile_pool(name="stream", bufs=4) as stream,
            tc.tile_pool(name="hstream", bufs=4) as hstream,
            tc.tile_pool(name="sums", bufs=3) as sums,
            tc.tile_pool(name="tanhs", bufs=3) as tanhs,
            tc.tile_pool(name="smalls", bufs=8) as smalls,
            tc.tile_pool(name="ctxs", bufs=1) as ctxs,
            tc.tile_pool(name="scrp", bufs=3) as scrp,
            tc.tile_pool(name="ps_big", bufs=1, space="PSUM") as ps_big,
            tc.tile_pool(name="ps_terms", bufs=2, space="PSUM") as ps_terms,
            tc.tile_pool(name="ps_tot", bufs=2, space="PSUM") as ps_tot,
        ):
            # ---------------- setup ----------------
            # NOTE: all DMAs go through HWDGE (nc.sync / nc.scalar) — SWDGE
            # (gpsimd) descriptor generation deadlocks against concurrent DVE
            # activity on this part (hardware port-sharing hazard).
            wv_b = singles.tile([P, N], F16)
            nc.sync.dma_start(
                out=wv_b[:],
                in_=bass.AP(tensor=wv_d, offset=0, ap=[[0, P], [1, N]]),
            )
            bdec_b = singles.tile([BL, N], F32)
            nc.sync.dma_start(
                out=bdec_b[:],
                in_=bass.AP(tensor=bdec_d, offset=0, ap=[[0, BL], [1, N]]),
            )

            # lhsT for the rank-2 "terms" matmul: partition 0 = ones,
            # partition 1 = coverage rows (original L order); host-prepared.
            lhsT_cov = singles.tile([2, BL, L], BF16)
            nc.sync.dma_start(
                out=lhsT_cov[:],
                in_=lhst_d[:].rearrange("two (b l) -> two b l", b=BL),
            )

            # rhs for the terms matmul: partition 0 = s (per batch),
            # partition 1 = W_c (host-replicated per batch).  bf16.
            rhs_sw = singles.tile([2, BL, N], BF16)
            nc.sync.dma_start(
                out=rhs_sw[1:2, :, :],
                in_=wc_d[:].rearrange("one (b n) -> one b n", b=BL),
            )

            mask_t = singles.tile([P, BL * NCH], F32)
            nc.sync.dma_start(out=mask_t[:], in_=mask_d[:])
            covs_t = singles.tile([P, BL * NCH], F32)
            nc.sync.dma_start(out=covs_t[:], in_=covs_d[:])

            e_t = singles.tile([P, BL * NCH], F32)
            attn_t = singles.tile([P, BL * NCH], F32)
            # a2[:, col, 0] = bf16(attn), a2[:, col, 1] = bf16 residual —
            # an [128, 2] slice is the lhsT of the merged context matmul
            a2 = singles.tile([P, BL * NCH, 2], BF16)
            attn_lof = singles.tile([P, BL * NCH], F32)
            covn_t = singles.tile([P, BL * NCH], F32)

            e_scr = singles.tile([P, N], F16)  # dead output of the dot-accum
            ones_col = singles.tile([P, 1], F32)
            nc.vector.memset(ones_col[:], 1.0)
            ones_row = singles.tile([1, P], F32)
            nc.vector.memset(ones_row[:], 1.0)

            # ---------------- dec_proj: s = s_t_hat @ W_dec.T + b_dec -------
            s_ps = ps_big.tile([BL, N], F32, tag="big")
            for kb in range(N // P):
                stT_t = smalls.tile([P, BL], BF16, tag="stT")
                nc.sync.dma_start(out=stT_t[:], in_=stT_d[kb * P : (kb + 1) * P, :])
                wdecT_t = wdec_pool.tile([P, N], BF16)
                nc.sync.dma_start(
                    out=wdecT_t[:], in_=wdecT_d[kb * P : (kb + 1) * P, :]
                )
                for nh in range(2):
                    nc.tensor.matmul(
                        s_ps[:, nh * 512 : (nh + 1) * 512],
                        stT_t[:],
                        wdecT_t[:, nh * 512 : (nh + 1) * 512],
                        start=(kb == 0),
                        stop=(kb == N // P - 1),
                    )
            s_sb = singles.tile([BL, N], F32)
            nc.vector.tensor_add(s_sb[:], s_ps[:], bdec_b[:])
            s_bf = singles.tile([BL, N], BF16)
            nc.vector.tensor_copy(s_bf[:], s_sb[:])
            for b in range(BL):
                # cross-partition move b -> 0 into the rhs tile
                nc.sync.dma_start(out=rhs_sw[0:1, b, :], in_=s_bf[b : b + 1, :])

            # ------------- staggered pipeline over local batches -------------
            # Stage s runs pass B of batch s interleaved (at the t-step
            # level) with pass C of batch s-1, so the DVE-bound score pass
            # and the PE-bound context pass overlap and the two DMA streams
            # advance together.
            ctx_ps = None
            for s in range(BL + 1):
                bB = s            # batch for pass B this stage
                bC = s - 1        # batch for pass C this stage
                if bC >= 0:
                    ctx_ps = ps_big.tile([2, N], F32, tag="big")
                pend_th = None
                for t in range(NT):
                    if bB < BL:
                        enc_t = stream.tile([P, Q, N], F16, tag="stream")
                        dme = nc.sync if t % 2 == 0 else nc.scalar
                        dme.dma_start(out=enc_t[:], in_=enc_r[bB, t])
                        su = sums.tile([P, Q, N], F16, tag="su")
                        for q in range(Q):
                            c = t * Q + q
                            terms = ps_terms.tile([P, N], F32, tag="terms")
                            for nh in range(2):
                                nc.tensor.matmul(
                                    terms[:, nh * 512 : (nh + 1) * 512],
                                    lhsT_cov[:, bB, c * P : (c + 1) * P],
                                    rhs_sw[:, bB, nh * 512 : (nh + 1) * 512],
                                    start=True,
                                    stop=True,
                                )
                            nc.vector.tensor_add(
                                su[:, q, :], enc_t[:, q, :], terms[:]
                            )
                        th = tanhs.tile([P, Q, N], F16, tag="th")
                        nc.scalar.activation(th[:], su[:], AF.Tanh)
                        # dots for the PREVIOUS tile: keeps DVE busy during
                        # this tile's tanh latency (engine order is fixed at
                        # schedule time, so emit adds(t) before dots(t-1))
                        if pend_th is not None:
                            pth, pt = pend_th
                            for q in range(Q):
                                c = pt * Q + q
                                scr = scrp.tile([P, N], F16, tag="scr")
                                nc.vector.tensor_mul(scr[:], pth[:, q, :], wv_b[:])
                                nc.vector.tensor_scalar(
                                    out=e_scr[:],
                                    in0=scr[:],
                                    scalar1=1.0,
                                    scalar2=0.0,
                                    op0=ALU.mult,
                                    op1=ALU.add,
                                    accum_out=e_t[:, bB * NCH + c : bB * NCH + c + 1],
                                )
                        pend_th = (th, t)

                    if bC >= 0:
                        # pass C t-step for batch bC (QH == Q so t aligns)
                        hh_t = hstream.tile([P, QH, N], BF16, tag="hstream")
                        nc.sync.dma_start(out=hh_t[:], in_=hh_r[bC, t])
                        hl_t = hstream.tile([P, QH, N], BF16, tag="hstream")
                        nc.sync.dma_start(out=hl_t[:], in_=hl_r[bC, t])
                        for q in range(QH):
                            c = t * QH + q
                            col = bC * NCH + c
                            for hi, h_til in enumerate((hh_t, hl_t)):
                                for nh in range(2):
                                    nsl = slice(nh * 512, (nh + 1) * 512)
                                    nc.tensor.matmul(
                                        ctx_ps[:, nsl],
                                        a2[:, col, :],
                                        h_til[:, q, nsl],
                                        start=(c == 0 and hi == 0),
                                        stop=(c == NCH - 1 and hi == 1),
                                        skip_group_check=True,
                                    )

                if bC >= 0:
                    # finalize context for bC: rows are hi/lo partial sums
                    ctx_sb = ctxs.tile([2, N], F32, tag="ctx")
                    nc.vector.tensor_copy(ctx_sb[:], ctx_ps[:])
                    ctx_red = ctxs.tile([2, N], F32, tag="ctxred")
                    nc.gpsimd.partition_all_reduce(
                        ctx_red[:],
                        ctx_sb[:],
                        channels=2,
                        reduce_op=bass_isa.ReduceOp.add,
                    )
                    nc.sync.dma_start(out=ctx_o[bC : bC + 1, :], in_=ctx_red[0:1, :])

                if bB < BL:
                    # flush the last tile's pending dots
                    if pend_th is not None:
                        pth, pt = pend_th
                        for q in range(Q):
                            c = pt * Q + q
                            scr = scrp.tile([P, N], F16, tag="scr")
                            nc.vector.tensor_mul(scr[:], pth[:, q, :], wv_b[:])
                            nc.vector.tensor_scalar(
                                out=e_scr[:],
                                in0=scr[:],
                                scalar1=1.0,
                                scalar2=0.0,
                                op0=ALU.mult,
                                op1=ALU.add,
                                accum_out=e_t[:, bB * NCH + c : bB * NCH + c + 1],
                            )
                        pend_th = None
                    # ---- masked softmax + renorm + coverage update for bB ----
                    sl = slice(bB * NCH, (bB + 1) * NCH)
                    pexp = smalls.tile([P, NCH], F32, tag="pexp")
                    # e was accumulated against W_v * 2^10; undo via Exp scale
                    nc.scalar.activation(
                        pexp[:], e_t[:, sl], AF.Exp, scale=1.0 / WV_SCALE
                    )
                    pm = smalls.tile([P, NCH], F32, tag="pm")
                    partial = smalls.tile([P, 1], F32, tag="partial")
                    nc.vector.scalar_tensor_tensor(
                        out=pm[:],
                        in0=pexp[:],
                        scalar=1.0,
                        in1=mask_t[:, sl],
                        op0=ALU.mult,
                        op1=ALU.mult,
                        accum_out=partial[:],
                    )
                    # partition-sum of `partial` + broadcast via two tiny
                    # fp32 matmuls on a dedicated PSUM bank
                    tot1 = ps_tot.tile([1, 1], F32, tag="tot")
                    nc.tensor.matmul(
                        tot1[:], partial[:], ones_col[:], start=True, stop=True
                    )
                    tot1_sb = smalls.tile([1, 1], F32, tag="tot1")
                    nc.vector.tensor_copy(tot1_sb[:], tot1[:])
                    tot2 = ps_tot.tile([P, 1], F32, tag="tot")
                    nc.tensor.matmul(
                        tot2[:], ones_row[:], tot1_sb[:], start=True, stop=True
                    )
                    rtot = smalls.tile([P, 1], F32, tag="rtot")
                    nc.vector.reciprocal(rtot[:], tot2[:])
                    nc.vector.tensor_scalar_mul(attn_t[:, sl], pm[:], rtot[:])
                    nc.vector.tensor_add(
                        covn_t[:, sl], covs_t[:, sl], attn_t[:, sl]
                    )
                    # attn hi/lo bf16 split for the context matmul
                    nc.vector.tensor_copy(a2[:, sl, 0], attn_t[:, sl])
                    nc.vector.tensor_sub(
                        attn_lof[:, sl], attn_t[:, sl], a2[:, sl, 0]
                    )
                    nc.vector.tensor_copy(a2[:, sl, 1], attn_lof[:, sl])
                    # stream per-batch outputs out as soon as they're ready
                    nc.sync.dma_start(out=attn_o[:, sl], in_=attn_t[:, sl])
                    nc.sync.dma_start(out=covn_o[:, sl], in_=covn_t[:, sl])

    nc.finalize()
    return nc


def _swz(x):
    """[BL, L] -> [P, BL*NCH]: column b*NCH+c, partition p <- x[b, c*P+p]."""
    return np.ascontiguousarray(
        x.reshape(BL, NCH, P).transpose(2, 0, 1).reshape(P, BL * NCH)
    )


def _unswz(y):
    """inverse of _swz"""
    return np.ascontiguousarray(
        y.reshape(P, BL, NCH).transpose(1, 2, 0).reshape(BL, L)
    )


def build_in_maps(inputs):
    return _build_in_maps(**inputs)


def _build_in_maps(h, enc_feat, attn_mask, s_t_hat, coverage, W_dec, b_dec, W_c, W_v):
    h = np.asarray(h, np.float32)
    enc_feat = np.asarray(enc_feat, np.float32)
    attn_mask = np.asarray(attn_mask, np.float32)
    s_t_hat = np.asarray(s_t_hat, np.float32)
    coverage = np.asarray(coverage, np.float32)
    h_hi = h.astype(NPBF16)
    h_lo = (h - h_hi.astype(np.float32)).astype(NPBF16)
    wdecT = np.ascontiguousarray(np.asarray(W_dec, np.float32).T).astype(NPBF16)
    bdec = np.ascontiguousarray(np.asarray(b_dec, np.float32).reshape(1, N))
    wc = np.tile(
        np.asarray(W_c, np.float32).reshape(1, N), (1, BL)
    ).reshape(1, BL * N).astype(NPBF16)
    wv = (np.asarray(W_v, np.float32).reshape(1, N) * WV_SCALE).astype(np.float16)

    in_maps = []
    for core in range(M):
        sl = slice(core * BL, (core + 1) * BL)
        cov = coverage[sl]
        lhst = np.concatenate(
            [np.ones((1, BL * L), np.float32), cov.reshape(1, BL * L)]
        ).astype(NPBF16)
        in_maps.append(
            {
                "h_hi": np.ascontiguousarray(h_hi[sl]),
                "h_lo": np.ascontiguousarray(h_lo[sl]),
                "enc": np.ascontiguousarray(enc_feat[sl].astype(np.float16)),
                "lhst": lhst,
                "cov_swz": _swz(cov),
                "mask_swz": _swz(attn_mask[sl]),
                "stT": np.ascontiguousarray(s_t_hat[sl].T.astype(NPBF16)),
                "wdecT": wdecT,
                "b_dec": bdec,
                "w_c": wc,
                "w_v": wv,
            }
        )
    return in_maps


def kernel(**inputs):
    global _CACHED_NC
    in_maps = build_in_maps(inputs)
    if _CACHED_NC is None:
        _CACHED_NC = _build_nc()
    res = run_bass_kernel_spmd(_CACHED_NC, in_maps, list(range(M)))

    attn = np.empty((B, L), np.float32)
    ctx = np.empty((B, N), np.float32)
    covn = np.empty((B, L), np.float32)
    for core in range(M):
        r = res.results[core]
        sl = slice(core * BL, (core + 1) * BL)
        attn[sl] = _unswz(r["attn_swz"])
        ctx[sl] = r["ctx"]
        covn[sl] = _unswz(r["covnew_swz"])
    return attn, ctx, covn
